# revision 22
# baseline (speedup 1.0000x reference)
"""Trainium2 Bass kernel for nn_DeepSeekNeuralMLP (SwiGLU MLP with
Catmull-Rom-spline-reconstructed weights), tensor-parallel over 8 NeuronCores.

Strategy (Megatron-style):
  - gate/up weights [8192, 2048] sharded over the intermediate dim: core r owns
    rows [r*1024, (r+1)*1024).  down weight [2048, 8192] sharded over its input
    (intermediate) dim: core r owns columns [r*1024, (r+1)*1024).  Each core
    produces a partial output [2048, 8192] (h-major, bf16); the host sums the 8
    partials in f32 and transposes to the final [4, 2048, 2048].
  - All three weight shards are generated on-device in bf16 and stay resident
    in SBUF, so each 512-token block flows gate/up -> SwiGLU -> down without
    spilling the intermediate activations to DRAM.
  - Spline reconstruction: the static sampling grid factors into 128-sample
    chunks; within a chunk the control interval index takes at most two values
    (j_c, j_c+1), so each chunk is two cubic evaluations blended by a static
    mask.  The host precomputes z = (u-powers x gathered cp taps) [21, CPB]
    directly (pure static-index layout prep), so the device does just: two
    small matmuls per 512-chunk block with static Vandermonde-style lhsT
    (VA/VB) -> copy + masked select into the bf16 weight tile.
  - Chunk orderings are chosen so every main-matmul lhsT slice is contiguous
    in SBUF (gate/up: h-block-major, down: i-block-major).
"""
import numpy as np
from math import comb

import concourse.bass as bass
from concourse import bacc, tile, mybir
from concourse.bass_utils import run_bass_kernel_spmd
import ml_dtypes

# ----------------------------------------------------------------------------
# static problem geometry (hardcoded; must match the reference)
# ----------------------------------------------------------------------------
HIDDEN = 2048
INTER = 8192
NTOK = 8192                    # 4 * 2048 tokens
NCORES = 8
N = INTER * HIDDEN             # samples per weight (same for all three)
NCTRL = max(16, int(N / 128.9))
NCHUNK = N // 128
CPB = NCHUNK // NCORES         # 16384 chunks per core per weight
IC = INTER // NCORES           # 1024 intermediate per core

F32 = mybir.dt.float32
F32R = mybir.dt.float32r
BF16 = mybir.dt.bfloat16
U8 = mybir.dt.uint8

_B_COEF = 0.5 * np.array([
    [0.0, -1.0,  2.0, -1.0],
    [2.0,  0.0, -5.0,  3.0],
    [0.0,  1.0,  4.0, -3.0],
    [0.0,  0.0, -1.0,  1.0],
], dtype=np.float64)           # Catmull-Rom basis b_t(f) coeffs, [tap, power]


def _static_tables():
    t = np.linspace(0.0, NCTRL - 1.0, N, dtype=np.float64)
    i = np.clip(np.floor(t).astype(np.int64), 0, NCTRL - 2)
    k0 = np.arange(NCHUNK, dtype=np.int64) * 128
    j = i[k0]
    iv = i.reshape(NCHUNK, 128)
    m = (iv == j[:, None]).sum(axis=1)
    u = t[k0] - j
    delta = (NCTRL - 1.0) / (N - 1.0)
    return j, u, m, delta


_J, _U, _M, _DELTA = _static_tables()


def _bderiv(y):
    y = np.asarray(y, dtype=np.float64)
    out = np.zeros((4, 4) + y.shape, dtype=np.float64)
    for e in range(4):
        for tp in range(4):
            for p in range(e, 4):
                out[e, tp] += comb(p, e) * _B_COEF[tp, p] * y ** (p - e)
    return out


def _va_vb():
    """Row map: z = (e-1)*5 + tau for e in 1..3 (rows 0..14), z = 15 dummy
    zero row, z = 16 + tau for e = 0 (raw cp tap rows)."""
    s = np.arange(128, dtype=np.float64)
    dA = _bderiv(s * _DELTA)
    dB = _bderiv(s * _DELTA - 1.0)
    VA = np.zeros((21, 128), dtype=np.float64)
    VB = np.zeros((21, 128), dtype=np.float64)
    for e in range(4):
        for tp in range(4):
            zA = 16 + tp if e == 0 else (e - 1) * 5 + tp
            zB = 16 + (tp + 1) if e == 0 else (e - 1) * 5 + (tp + 1)
            VA[zA] = dA[e, tp]
            VB[zB] = dB[e, tp]
    return VA.astype(np.float32), VB.astype(np.float32)


_VA, _VB = _va_vb()


def _chunklists():
    """Chunk->SBUF-column orderings chosen for contiguous matmul lhsT slices
    AND so that generation order matches first-use order in the main loop.

    gate/up weight [INTER, HIDDEN]: global chunk g = i*16 + hb (hb = h/128).
    Column c' = it*2048 + kt*128 + q (it-major), i = r*1024 + it*128 + q,
    hb = kt.  lhsT for (it, kt) is columns [it*2048 + kt*128, +128) --
    contiguous, and B(it) only needs columns [it*2048, (it+1)*2048).

    down weight [HIDDEN, INTER]: global chunk g = h*64 + r*8 + ib.  Column
    c' = ht*1024 + ib*128 + q (ht-major), h = ht*128 + q.  lhsT for (ht, ib)
    is columns [ht*1024 + ib*128, +128) -- contiguous, and D(ht) only needs
    columns [ht*1024, (ht+1)*1024).
    """
    c = np.arange(CPB, dtype=np.int64)
    gateup = np.empty((NCORES, CPB), dtype=np.int64)
    down = np.empty((NCORES, CPB), dtype=np.int64)
    it, kt, q = c // 2048, (c % 2048) // 128, c % 128
    ht, ib = c // 1024, (c % 1024) // 128
    for r in range(NCORES):
        gateup[r] = (r * IC + it * 128 + q) * 16 + kt
        down[r] = (ht * 128 + q) * 64 + r * 8 + ib
    return gateup, down


_CL_GU, _CL_DN = _chunklists()


def _static_for_clist(cl):
    """cp gather indices [5, CPB], u powers [3, CPB], mask [128, CPB]."""
    j = _J[cl]
    u = _U[cl]
    m = _M[cl]
    idx = np.clip(j[None, :] + np.arange(-1, 4)[:, None], 0, NCTRL - 1)
    up = np.stack([u, u * u, u * u * u]).astype(np.float32)
    s = np.arange(128, dtype=np.int64)
    mask = (s[:, None] >= m[None, :]).astype(np.uint8)
    return idx, up, np.ascontiguousarray(mask)


_STATIC_GU = [_static_for_clist(_CL_GU[r]) for r in range(NCORES)]
_STATIC_DN = [_static_for_clist(_CL_DN[r]) for r in range(NCORES)]


def _z_for(cp, idx, upow):
    """Host-side z build [21, CPB]: rows 0-14 = u^e * taps (e=1..3), row 15
    zero, rows 16-20 = raw taps.  Pure static-index gather + broadcast mul."""
    rows = np.take(cp, idx).astype(np.float32)          # [5, CPB]
    z = np.zeros((21, idx.shape[1]), dtype=np.float32)
    z[16:21] = rows
    for e in (1, 2, 3):
        z[(e - 1) * 5:(e - 1) * 5 + 5] = rows * upow[e - 1][None, :]
    return z


# ----------------------------------------------------------------------------
# device program
# ----------------------------------------------------------------------------
def _build_program():
    nc = bacc.Bacc("TRN2", target_bir_lowering=False, debug=False,
                   num_devices=NCORES)

    hsT = nc.dram_tensor("hsT", [HIDDEN, NTOK], BF16, kind="ExternalInput")
    va_d = nc.dram_tensor("va", [21, 128], F32R, kind="ExternalInput")
    vb_d = nc.dram_tensor("vb", [21, 128], F32R, kind="ExternalInput")
    z_d = {w: nc.dram_tensor(f"z_{w}", [21, CPB], F32R, kind="ExternalInput")
           for w in ("gate", "up", "down")}
    mask_gu_d = nc.dram_tensor("mask_gu", [128, CPB], U8, kind="ExternalInput")
    mask_dn_d = nc.dram_tensor("mask_dn", [128, CPB], U8, kind="ExternalInput")
    outT = nc.dram_tensor("outT", [HIDDEN, NTOK], BF16, kind="ExternalOutput")

    with tile.TileContext(nc) as tc:
        import contextlib
        with contextlib.ExitStack() as ctx:
            pools = {
                "const": ctx.enter_context(tc.tile_pool(name="const", bufs=1)),
                "wgt": ctx.enter_context(tc.tile_pool(name="wgt", bufs=1)),
                "zp": ctx.enter_context(tc.tile_pool(name="zp", bufs=6)),
                "mk": ctx.enter_context(tc.tile_pool(name="mk", bufs=6)),
                "hs": ctx.enter_context(tc.tile_pool(name="hs", bufs=32)),
                "sil": ctx.enter_context(tc.tile_pool(name="sil", bufs=4)),
                "inter": ctx.enter_context(tc.tile_pool(name="inter", bufs=16)),
                "ot": ctx.enter_context(tc.tile_pool(name="ot", bufs=4)),
                "psB": ctx.enter_context(
                    tc.tile_pool(name="psB", bufs=4, space="PSUM")),
                "psG": ctx.enter_context(
                    tc.tile_pool(name="psG", bufs=4, space="PSUM")),
            }
            va_t = pools["const"].tile([21, 128], F32R, tag="va")
            vb_t = pools["const"].tile([21, 128], F32R, tag="vb")
            nc.sync.dma_start(va_t[:], va_d[:])
            nc.sync.dma_start(vb_t[:], vb_d[:])

            # HAM warm-up: throwaway matmuls on zeroed tiles fill the initial
            # input-DMA window with continuous PE activity, so the clock gate
            # opens (1.2 -> 2.4 GHz) before the first real matmul arrives.
            wz_l = pools["const"].tile([128, 128], BF16, tag="wzl")
            wz_r = pools["const"].tile([128, 512], BF16, tag="wzr")
            nc.vector.memset(wz_l[:], 0)
            nc.vector.memset(wz_r[:], 0)
            for _ in range(12):
                pw = pools["psG"].tile([128, 512], F32, tag="gn")
                nc.tensor.matmul(pw[:], wz_l[:], wz_r[:], start=True,
                                 stop=True)

            wt = {w: pools["wgt"].tile([128, CPB], BF16, tag=w, name=w)
                  for w in ("gate", "up", "down")}

            # ---- weight generation, one 1024-column unit at a time.  Units
            # are emitted just-in-time ahead of their first consumer in the
            # tb=0 main-loop pass, so the PE never sits idle on gen epilogues.
            def gen_fetch(w, mask_dram, u, split=False):
                zt = pools["zp"].tile([21, 1024], F32R, tag="z")
                if split:
                    # halves, so the first matmul only waits on 512 columns
                    nc.sync.dma_start(
                        zt[:, 0:512], z_d[w][:, u * 1024:u * 1024 + 512])
                    nc.sync.dma_start(
                        zt[:, 512:1024],
                        z_d[w][:, u * 1024 + 512:(u + 1) * 1024])
                else:
                    nc.sync.dma_start(
                        zt[:], z_d[w][:, u * 1024:(u + 1) * 1024])
                mt = pools["mk"].tile([128, 1024], U8, tag="m")
                nc.sync.dma_start(mt[:], mask_dram[:, u * 1024:(u + 1) * 1024])
                return zt, mt

            def gen_compute(w, u, zt, mt):
                for b in range(2):              # 512-chunk blocks
                    zsl = zt[:, b * 512:(b + 1) * 512]
                    pa = pools["psG"].tile([128, 512], F32, tag="gn")
                    pb = pools["psG"].tile([128, 512], F32, tag="gn")
                    nc.tensor.matmul(pa[:], va_t[:], zsl, start=True,
                                     stop=True)
                    nc.tensor.matmul(pb[:], vb_t[:], zsl, start=True,
                                     stop=True)
                    col = u * 1024 + b * 512
                    wsl = wt[w][:, col:col + 512]
                    nc.scalar.copy(wsl, pa[:])
                    nc.vector.copy_predicated(
                        wsl, mt[:, b * 512:(b + 1) * 512], pb[:])

            def gen_unit(w, mask_dram, u, split=False):
                gen_compute(w, u, *gen_fetch(w, mask_dram, u, split))

            # gen schedule for tb=0: which unit to emit before each B-chain
            # k-tile group (4 groups per it) and each D chain.  B(0,it) needs
            # gate/up units {2it, 2it+1}; D(0,ht) needs down unit ht.  Every
            # slot stays >=1 full chain ahead of its consumer.
            gu_units = [("gate", 0), ("gate", 1)]
            # tb0 B(0,it) runs pg then pu sequentially; the pg chain embeds
            # this it's up-units (consumed by the following pu chain) and the
            # pu chain embeds the next it's gate-units.
            pg_slots = {}
            pu_slots = {}
            for it in range(8):
                pg_slots[(it, 0)] = ("up", 2 * it)
                pg_slots[(it, 8)] = ("up", 2 * it + 1)
                if it < 7:
                    pu_slots[(it, 0)] = ("gate", 2 * it + 2)
                    pu_slots[(it, 8)] = ("gate", 2 * it + 3)
            pu_slots[(7, 0)] = ("down", 0)
            pu_slots[(7, 8)] = ("down", 1)
            d_slots = {0: [("down", 2), ("down", 3)]}
            for ht in range(1, 13):
                d_slots[ht] = [("down", ht + 3)]

            # ---- main loop: per 512-token block, gate/up -> SwiGLU -> down.
            # tb=0 interleaves the weight generation.
            for tb in range(16):
                prefetched = {}
                if tb == 0:
                    for k, (w, u) in enumerate(gu_units):
                        gen_unit(w, mask_gu_d, u, split=(k == 0))
                    # issue it=0's gen DMAs ahead of the 2MB hs block so the
                    # single DMA queue doesn't starve the first B iteration
                    for key in (("pg", 0, 0), ("pg", 0, 8),
                                ("pu", 0, 0), ("pu", 0, 8)):
                        w, u = (pg_slots if key[0] == "pg"
                                else pu_slots)[key[1:]]
                        prefetched[key] = gen_fetch(w, mask_gu_d, u)
                hs_tiles = []
                for kt in range(16):
                    t = pools["hs"].tile([128, 512], BF16, tag="t")
                    nc.sync.dma_start(
                        t[:], hsT[kt * 128:(kt + 1) * 128,
                                  tb * 512:(tb + 1) * 512])
                    hs_tiles.append(t)
                int_tiles = []
                for it in range(8):
                    def _slot(tbl, key):
                        w, u = tbl[key[1:]]
                        if key in prefetched:
                            gen_compute(w, u, *prefetched[key])
                        else:
                            gen_unit(w, mask_dn_d if w == "down"
                                     else mask_gu_d, u)
                    pg = pools["psB"].tile([128, 512], F32, tag="ps")
                    pu = pools["psB"].tile([128, 512], F32, tag="ps")
                    if tb == 0:
                        # sequential pg then pu chains with embedded gen units
                        for kt in range(16):
                            if kt in (0, 8):
                                _slot(pg_slots, ("pg", it, kt))
                            base = it * 2048 + kt * 128
                            nc.tensor.matmul(
                                pg[:], wt["gate"][:, base:base + 128],
                                hs_tiles[kt][:],
                                start=(kt == 0), stop=(kt == 15))
                        sil = pools["sil"].tile([128, 512], F32, tag="sil")
                        nc.scalar.activation(
                            sil[:], pg[:], mybir.ActivationFunctionType.Silu)
                        for kt in range(16):
                            if kt in (0, 8):
                                _slot(pu_slots, ("pu", it, kt))
                            base = it * 2048 + kt * 128
                            nc.tensor.matmul(
                                pu[:], wt["up"][:, base:base + 128],
                                hs_tiles[kt][:],
                                start=(kt == 0), stop=(kt == 15))
                    else:
                        for kt in range(16):
                            base = it * 2048 + kt * 128
                            lg = wt["gate"][:, base:base + 128]
                            lu = wt["up"][:, base:base + 128]
                            rhs = hs_tiles[kt][:]
                            nc.tensor.matmul(pg[:], lg, rhs,
                                             start=(kt == 0), stop=(kt == 15))
                            nc.tensor.matmul(pu[:], lu, rhs,
                                             start=(kt == 0), stop=(kt == 15))
                        sil = pools["sil"].tile([128, 512], F32, tag="sil")
                        nc.scalar.activation(
                            sil[:], pg[:], mybir.ActivationFunctionType.Silu)
                    itile = pools["inter"].tile([128, 512], BF16, tag="it")
                    nc.vector.tensor_mul(itile[:], sil[:], pu[:])
                    int_tiles.append(itile)
                for ht in range(16):
                    if tb == 0:
                        for w, u in d_slots.get(ht, ()):
                            gen_unit(w, mask_dn_d, u)
                    pd = pools["psB"].tile([128, 512], F32, tag="ps")
                    for ib in range(8):
                        base = ht * 1024 + ib * 128
                        ld = wt["down"][:, base:base + 128]
                        nc.tensor.matmul(pd[:], ld, int_tiles[ib][:],
                                         start=(ib == 0), stop=(ib == 7))
                    o = pools["ot"].tile([128, 512], BF16, tag="ot")
                    if ht % 2 == 0:
                        nc.scalar.copy(o[:], pd[:])
                    else:
                        nc.vector.tensor_copy(o[:], pd[:])
                    nc.sync.dma_start(
                        outT[ht * 128:(ht + 1) * 128,
                             tb * 512:(tb + 1) * 512], o[:])

    nc.compile()
    return nc


_NC_CACHE = None


def _get_program():
    global _NC_CACHE
    if _NC_CACHE is None:
        _NC_CACHE = _build_program()
    return _NC_CACHE


def _in_maps(hidden_states, gate_cp, up_cp, down_cp):
    hs = np.ascontiguousarray(
        np.asarray(hidden_states, dtype=np.float32)
        .reshape(NTOK, HIDDEN).T).astype(ml_dtypes.bfloat16)
    cps = {"gate": np.asarray(gate_cp, dtype=np.float32),
           "up": np.asarray(up_cp, dtype=np.float32),
           "down": np.asarray(down_cp, dtype=np.float32)}
    maps = []
    for r in range(NCORES):
        idx_gu, up_gu, mask_gu = _STATIC_GU[r]
        idx_dn, up_dn, mask_dn = _STATIC_DN[r]
        m = {"hsT": hs, "va": _VA, "vb": _VB,
             "mask_gu": mask_gu, "mask_dn": mask_dn}
        for w in ("gate", "up", "down"):
            idx, upow = (idx_dn, up_dn) if w == "down" else (idx_gu, up_gu)
            m[f"z_{w}"] = _z_for(cps[w], idx, upow)
        maps.append(m)
    return maps


def kernel(hidden_states, gate_cp, up_cp, down_cp, _trace=False):
    nc = _get_program()
    maps = _in_maps(hidden_states, gate_cp, up_cp, down_cp)
    res = run_bass_kernel_spmd(nc, maps, core_ids=list(range(NCORES)),
                               trace=_trace)
    out_T = np.zeros((HIDDEN, NTOK), dtype=np.float32)
    for r in range(NCORES):
        out_T += res.results[r]["outT"].astype(np.float32)
    out = np.ascontiguousarray(out_T.T).reshape(4, 2048, HIDDEN)
    if _trace:
        kernel.last_results = res
    return out
